# revision 62
# baseline (speedup 1.0000x reference)
"""Bass/Trainium2 kernel for BiLinearLayer.

reference math (per batch b):
    att = relu(q1 @ U @ q2^T)            [T1, T2]
    w1  = softmax(att, axis=T1)          (column softmax)
    w2  = softmax(att, axis=T2)          (row softmax)
    q1_align = w1^T @ q1                 [T2, D]
    q2_align = w2 @ q2                   [T1, D]
returns (q1_align, q2_align), each [B, T, D] float32.

Sharding: data-parallel over batch B across 8 NeuronCores (8 batches/core),
U replicated.

Precision: the two big matmuls (q1@U and P@q2^T) run single-pass fp32r
(1 cycle/row on TRN2 for free-dim >= 256, same speed as bf16; products
round to ~fp22). Operands are pre-rounded to fp22 RNE on the host so HW
operand rounding is a no-op. The align matmuls run bf16 (E weights from
exp() rounded to bf16, q rounded to bf16; products exact in the fp32
accumulator); softmax normalization is deferred to the PSUM->SBUF output
copy via per-partition 1/sum scales, so weight-rounding error stays at the
bf16 level. att is kept in full fp32; its transpose (for the column
softmax) runs in exact fp32 PE-transpose mode. Outputs store as bf16
(halves store-drain latency that was stalling the out-tile ring) and the
host upconverts to fp32.

Schedule per batch i: P^T(i) matmul groups with batch i-1's E-transpose
groups (bf16 PE transposes, 1c/row) interleaved between them; att(i)
groups with relu/max/exp fused per block; aligns(i-1); attT(i) + column
softmax. U loads as row-block DMAs (contiguous 4KB segments; the
column-chunked rearrange costs ~10us of descriptor gen) and batch 0's
P^T runs its first 6 psum groups db-major so compute starts as soon as
the first q1t/U chunks land. All align scales run on DVE so store
descriptor pushes on the scalar queue never delay the psum-freeing
copies. Input loads ride the sync DMA ring; output stores and U ride the
scalar ring (tail stores switch to sync once loads are done). DMA-XBAR
transposes were tried and rejected: scalar-ring dma_start_transpose
returns corrupt data on HW (sync-ring ones were correct), and stores
behind prefetch loads on the sync ring stall the out-tile ring.
"""

import sys

if "/opt/trn_rl_repo" not in sys.path:
    sys.path.insert(0, "/opt/trn_rl_repo")

from contextlib import ExitStack

import numpy as np

import concourse.bass as bass
import concourse.mybir as mybir
import concourse.tile as tile
from concourse import bacc
from concourse.masks import make_identity

F32 = mybir.dt.float32
F32R = mybir.dt.float32r
BF16 = mybir.dt.bfloat16
AF = mybir.ActivationFunctionType
AX = mybir.AxisListType

B, T, D = 64, 512, 1024
NCORES = 8
BL = B // NCORES  # batches per core
P = 128
TB = T // P  # 4 t/s blocks
DB = D // P  # 8 d/e blocks


def build_nc():
    nc = bacc.Bacc()
    q1td = nc.dram_tensor("q1t", [BL, D, T], F32R, kind="ExternalInput")
    q2td = nc.dram_tensor("q2t", [BL, D, T], F32R, kind="ExternalInput")
    n1d = nc.dram_tensor("n1", [BL, T, D], BF16, kind="ExternalInput")
    n2d = nc.dram_tensor("n2", [BL, T, D], BF16, kind="ExternalInput")
    ud = nc.dram_tensor("u", [D, D], F32R, kind="ExternalInput")
    o1 = nc.dram_tensor("o1", [BL, T, D], BF16, kind="ExternalOutput")
    o2 = nc.dram_tensor("o2", [BL, T, D], BF16, kind="ExternalOutput")

    with tile.TileContext(nc) as tc, ExitStack() as ctx:
        const = ctx.enter_context(tc.tile_pool(name="const", bufs=1))
        qt_p = ctx.enter_context(tc.tile_pool(name="qt", bufs=4))
        n_p = ctx.enter_context(tc.tile_pool(name="n", bufs=4))
        pt_p = ctx.enter_context(tc.tile_pool(name="pt", bufs=1))
        att_p = ctx.enter_context(tc.tile_pool(name="att", bufs=1))
        e_p = ctx.enter_context(tc.tile_pool(name="e", bufs=1))
        st_p = ctx.enter_context(tc.tile_pool(name="st", bufs=4))
        out_p = ctx.enter_context(tc.tile_pool(name="out", bufs=6))
        ps_mm = ctx.enter_context(tc.tile_pool(name="ps_mm", bufs=4, space="PSUM"))
        ps_trf = ctx.enter_context(tc.tile_pool(name="ps_trf", bufs=2, space="PSUM"))
        ps_trb = ctx.enter_context(tc.tile_pool(name="ps_trb", bufs=2, space="PSUM"))

        ident_f32 = const.tile([P, P], F32)
        make_identity(nc, ident_f32[:])
        ident_bf = const.tile([P, P], BF16)
        nc.vector.tensor_copy(ident_bf[:], ident_f32[:])

        # U resident in fp32r, loaded as 8 row-block DMAs (contiguous 4KB
        # segments both sides; the column-chunked rearrange costs ~10us of
        # descriptor generation) on the scalar HWDGE ring so batch 0's input
        # loads (sync ring) run in parallel.
        u_sb = const.tile([P, DB, D], F32R)
        for db in range(DB):
            nc.scalar.dma_start(
                out=u_sb[:, db, :], in_=ud[db * P : (db + 1) * P, :]
            )

        def etrans_groups(st):
            """8 deferred PE groups: E2[t,s] -> E2T[s,t] and E1T[s,t] -> E1[t,s]
            (bf16, 1c/row). Emitted one per P^T psum-group slot of the next
            batch."""
            st["e2t"] = e_p.tile([P, TB, T], BF16, tag="e2t", name="e2t", bufs=2)
            st["e1"] = e_p.tile([P, TB, T], BF16, tag="e1", name="e1", bufs=2)
            groups = []

            def mk_e2t(sb):
                def g():
                    ps = ps_trb.tile([P, T], BF16, tag="pstrb", name="pstrb")
                    for tb in range(TB):
                        nc.tensor.transpose(
                            ps[:, tb * P : (tb + 1) * P],
                            st["e2"][:, tb, sb * P : (sb + 1) * P],
                            ident_bf[:],
                        )
                    nc.vector.tensor_copy(st["e2t"][:, sb, :], ps[:])

                return g

            def mk_e1(tb):
                def g():
                    ps = ps_trb.tile([P, T], BF16, tag="pstrb", name="pstrb")
                    for sb in range(TB):
                        nc.tensor.transpose(
                            ps[:, sb * P : (sb + 1) * P],
                            st["e1t"][:, sb, tb * P : (tb + 1) * P],
                            ident_bf[:],
                        )
                    nc.vector.tensor_copy(st["e1"][:, tb, :], ps[:])

                return g

            for sb in range(TB):
                groups.append(mk_e2t(sb))
            for tb in range(TB):
                groups.append(mk_e1(tb))
            return groups

        def pt_phase(i, ext_groups):
            """P^T[e,t] = sum_db U[db,e]^T q1^T[db,t], fp32r single pass.
            Batch 0 runs its first 6 psum groups db-major so compute starts as
            soon as the first q1t/U row-block chunks land. Later batches are
            eb-major with batch i-1's E-transpose groups interleaved."""
            gi = iter(ext_groups or [])
            q1ts = qt_p.tile([P, DB, T], F32R, tag="qt", name="q1ts")
            q1r = q1td[i].rearrange("(db p) t -> p db t", p=P)
            if i == 0:
                for db in range(DB):
                    nc.sync.dma_start(out=q1ts[:, db, :], in_=q1r[:, db, :])
            else:
                nc.sync.dma_start(out=q1ts[:], in_=q1r)
            q2ts = qt_p.tile([P, DB, T], F32R, tag="qt", name="q2ts")
            nc.sync.dma_start(
                out=q2ts[:], in_=q2td[i].rearrange("(db p) t -> p db t", p=P)
            )

            pt = pt_p.tile([P, DB, T], F32R, tag="pt", name="pt")
            if i == 0:
                NE = 6
                pss = [
                    (ps_mm if eb < 4 else ps_trf).tile(
                        [P, T], F32,
                        tag="psmm" if eb < 4 else "pstrf", name="ps0",
                    )
                    for eb in range(NE)
                ]
                for db in range(DB - 1):
                    for eb in range(NE):
                        nc.tensor.matmul(
                            pss[eb][:],
                            u_sb[:, db, eb * P : (eb + 1) * P],
                            q1ts[:, db, :],
                            start=(db == 0),
                            stop=False,
                        )
                for eb in range(NE):
                    # last db sweep: finish each group and copy immediately so
                    # the pt copies drain while later groups still compute
                    nc.tensor.matmul(
                        pss[eb][:],
                        u_sb[:, DB - 1, eb * P : (eb + 1) * P],
                        q1ts[:, DB - 1, :],
                        start=False,
                        stop=True,
                    )
                    nc.vector.tensor_copy(pt[:, eb, :], pss[eb][:])
                ebs = range(NE, DB)
            else:
                ebs = range(DB)
            for eb in ebs:
                ps = ps_mm.tile([P, T], F32, tag="psmm", name="psmm")
                for db in range(DB):
                    nc.tensor.matmul(
                        ps[:],
                        u_sb[:, db, eb * P : (eb + 1) * P],
                        q1ts[:, db, :],
                        start=(db == 0),
                        stop=(db == DB - 1),
                    )
                nc.vector.tensor_copy(pt[:, eb, :], ps[:])
                for g in gi:  # at most one deferred group per slot
                    g()
                    break
            return dict(pt=pt, q2ts=q2ts, gi=gi)

        def att_phase(i, st):
            """att[t,s] = relu(sum_eb P^T[eb,t]^T q2^T[eb,s]), fp32r; fused
            row softmax: E2 = exp(att - rowmax) bf16, r2 = 1/rowsum."""
            # n loads ride the scalar ring: on the sync ring they'd sit ahead
            # of the next batch's q-loads and delay q2t past the att phase.
            n1s = n_p.tile([P, TB, D], BF16, tag="n", name="n1s")
            nc.scalar.dma_start(
                out=n1s[:], in_=n1d[i].rearrange("(tb p) d -> p tb d", p=P)
            )
            n2s = n_p.tile([P, TB, D], BF16, tag="n", name="n2s")
            nc.scalar.dma_start(
                out=n2s[:], in_=n2d[i].rearrange("(tb p) d -> p tb d", p=P)
            )

            pt, q2ts = st["pt"], st["q2ts"]
            attr = att_p.tile([P, TB, T], F32, tag="attr", name="attr")
            e2 = e_p.tile([P, TB, T], BF16, tag="e2", name="e2")
            r2 = st_p.tile([P, TB], F32, tag="str2", name="r2", bufs=2)
            st.update(attr=attr, e2=e2, r2=r2, n1s=n1s, n2s=n2s)
            for tb in range(TB):
                ps = ps_mm.tile([P, T], F32, tag="psmm", name="psmm")
                for eb in range(DB):
                    nc.tensor.matmul(
                        ps[:],
                        pt[:, eb, tb * P : (tb + 1) * P],
                        q2ts[:, eb, :],
                        start=(eb == 0),
                        stop=(eb == DB - 1),
                    )
                nc.scalar.activation(attr[:, tb, :], ps[:], AF.Relu)
                nm = st_p.tile([P, 1], F32, tag="stm", name="nm2")
                nc.vector.reduce_max(
                    out=nm[:], in_=attr[:, tb, :], axis=AX.X,
                    op=mybir.AluOpType.max, negate=True,
                )
                sm = st_p.tile([P, 1], F32, tag="sts", name="sm2")
                nc.scalar.activation(
                    e2[:, tb, :], attr[:, tb, :], AF.Exp, bias=nm[:], accum_out=sm[:]
                )
                nc.vector.reciprocal(r2[:, tb : tb + 1], sm[:])

        def attT_phase(i, st):
            """attT[s,t] via fp32 PE transpose (exact); column softmax:
            E1T = exp(attT - colmax) bf16, r1 = 1/colsum."""
            attr = st["attr"]
            attT = att_p.tile([P, TB, T], F32, tag="attT", name="attT")
            e1t = e_p.tile([P, TB, T], BF16, tag="e1t", name="e1t")
            r1 = st_p.tile([P, TB], F32, tag="str1", name="r1", bufs=2)
            st.update(e1t=e1t, r1=r1)
            for sb in range(TB):
                ps = ps_trf.tile([P, T], F32, tag="pstrf", name="pstrf")
                for tb in range(TB):
                    nc.tensor.transpose(
                        ps[:, tb * P : (tb + 1) * P],
                        attr[:, tb, sb * P : (sb + 1) * P],
                        ident_f32[:],
                    )
                nc.scalar.copy(attT[:, sb, :], ps[:])
                nm = st_p.tile([P, 1], F32, tag="stm", name="nm1")
                nc.vector.reduce_max(
                    out=nm[:], in_=attT[:, sb, :], axis=AX.X,
                    op=mybir.AluOpType.max, negate=True,
                )
                sm = st_p.tile([P, 1], F32, tag="sts", name="sm1")
                nc.scalar.activation(
                    e1t[:, sb, :], attT[:, sb, :], AF.Exp, bias=nm[:], accum_out=sm[:]
                )
                nc.vector.reciprocal(r1[:, sb : sb + 1], sm[:])

        def aligns_phase(i, st, tail=False):
            """q2_align[t,d] = r2[t] * sum_sb E2T[sb,t]^T n2[sb,d] and
            q1_align[s,d] = r1[s] * sum_tb E1[tb,s]^T n1[tb,d], bf16."""
            e1, e2t = st["e1"], st["e2t"]
            r1, r2, n1s, n2s = st["r1"], st["r2"], st["n1s"], st["n2s"]
            for tb in range(TB):
                ob = out_p.tile([P, D], BF16, tag="out", name="ob2")
                for dh in range(2):
                    ps = ps_mm.tile([P, T], F32, tag="psmm", name="psmm")
                    for sb in range(TB):
                        nc.tensor.matmul(
                            ps[:],
                            e2t[:, sb, tb * P : (tb + 1) * P],
                            n2s[:, sb, dh * 512 : (dh + 1) * 512],
                            start=(sb == 0),
                            stop=(sb == TB - 1),
                        )
                    nc.vector.tensor_scalar_mul(
                        ob[:, dh * 512 : (dh + 1) * 512], ps[:], r2[:, tb : tb + 1]
                    )
                (nc.sync if tail else nc.scalar).dma_start(
                    out=o2[i, tb * P : (tb + 1) * P, :], in_=ob[:]
                )
            for sb in range(TB):
                ob = out_p.tile([P, D], BF16, tag="out", name="ob1")
                for dh in range(2):
                    ps = ps_mm.tile([P, T], F32, tag="psmm", name="psmm")
                    for tb in range(TB):
                        nc.tensor.matmul(
                            ps[:],
                            e1[:, tb, sb * P : (sb + 1) * P],
                            n1s[:, tb, dh * 512 : (dh + 1) * 512],
                            start=(tb == 0),
                            stop=(tb == TB - 1),
                        )
                    nc.vector.tensor_scalar_mul(
                        ob[:, dh * 512 : (dh + 1) * 512], ps[:],
                        r1[:, sb : sb + 1],
                    )
                (nc.sync if tail else nc.scalar).dma_start(
                    out=o1[i, sb * P : (sb + 1) * P, :], in_=ob[:]
                )

        groups = None
        states = {}
        for i in range(BL):
            st = pt_phase(i, groups)
            states[i] = st
            att_phase(i, st)
            if i > 0:
                aligns_phase(i - 1, states[i - 1])
            attT_phase(i, st)
            groups = etrans_groups(st)
        for g in groups:
            g()
        aligns_phase(BL - 1, states[BL - 1], tail=True)

    nc.compile()
    return nc


def _rne22(x):
    """Round fp32 to fp22 (13 explicit mantissa bits), RNE."""
    u = np.ascontiguousarray(x, dtype=np.float32).view(np.uint32)
    lsb = (u >> np.uint32(10)) & np.uint32(1)
    u2 = (u + np.uint32(0x1FF) + lsb) & np.uint32(0xFFFFFC00)
    return u2.view(np.float32)


def prep_inputs(q1, q2, U):
    """Host-side layout/precision prep shared by kernel() and test harness."""
    import ml_dtypes

    q1 = np.ascontiguousarray(q1, dtype=np.float32)
    q2 = np.ascontiguousarray(q2, dtype=np.float32)
    U = np.ascontiguousarray(U, dtype=np.float32)
    q1t = _rne22(np.ascontiguousarray(q1.transpose(0, 2, 1)))
    q2t = _rne22(np.ascontiguousarray(q2.transpose(0, 2, 1)))
    return {
        "q1t": q1t,
        "q2t": q2t,
        "n1": q1.astype(ml_dtypes.bfloat16),
        "n2": q2.astype(ml_dtypes.bfloat16),
        "u": _rne22(U),
    }


_NC_CACHE = None


def _get_nc():
    global _NC_CACHE
    if _NC_CACHE is None:
        _NC_CACHE = build_nc()
    return _NC_CACHE


def kernel(q1: np.ndarray, q2: np.ndarray, U: np.ndarray):
    from concourse import bass_utils

    nc = _get_nc()
    full = prep_inputs(q1, q2, U)
    in_maps = []
    for c in range(NCORES):
        s = slice(c * BL, (c + 1) * BL)
        in_maps.append(
            {k: (v if v.ndim == 2 else v[s]) for k, v in full.items()}
        )
    res = bass_utils.run_bass_kernel_spmd(nc, in_maps, list(range(NCORES)))
    o1 = np.concatenate(
        [np.asarray(res.results[c]["o1"], dtype=np.float32) for c in range(NCORES)],
        axis=0,
    )
    o2 = np.concatenate(
        [np.asarray(res.results[c]["o2"], dtype=np.float32) for c in range(NCORES)],
        axis=0,
    )
    return (o1, o2)
